# revision 15
# baseline (speedup 1.0000x reference)
"""CrossAttention Trainium2 Bass kernel (v5).

Full inputs in, full output out. Data-parallel over batch: 8 batch elements
-> 8 NeuronCores; each core runs the whole cross-attention for one batch
element. Weights are replicated; no collectives.

Per-core computation (transposed domain end-to-end):
  x [512, 4096] (c-major)  -> qT = Wq.T @ x            [512(i), 4096(t)]
  ctx [77, 768]            -> k/v = ctxT.T @ Wk/Wv     [77(j), 512(i)]
  per head pair p: simT pair in one 2-bank PSUM tile   [77(j), 2, t]
      (the two QK matmuls are row-tiled at partition bases 0/64 and run
      concurrently on the PE); one paired ACT exp evacuates both halves.
  pair-broadcast rowsums: two accumulating matmuls with all-ones selector
      stationaries (ones_lo: cols 0-63, ones_hi: cols 64-127) produce
      psrs_p[c, t] = rowsum_{2p + c//64}[t] directly in broadcast layout.
  bc_p = 1/psrs_p  (DVE reciprocal_approx_fast, PSUM -> SBUF)
  AV pairs: pav_p [128, TC] = zero-padded pair bank (2 accumulating MMs)
  ou_p = pav_p * bc_p  (fused DVE tensor_tensor; PSUM evac + softmax
      normalization in one op; per-pair tiles so the O projection's first
      matmul only waits on pair 0)
  y = Wo.T @ ou + bo                                   [512(c), 4096(t)]

All matmul operands are bf16; inputs are cast AND pre-permuted to the
on-chip [partition, free] layouts host-side, so every DMA moves fully
contiguous >=4KB lines per partition (the v4 strided rearrange reads ran
at ~1/3 DMA rate and delayed the first matmul to 12 us). The output is
written bf16 in a chunk-major layout and unscrambled host-side. A short
burst of scratch matmuls pre-warms the PE's HAM clock gate (cold PE runs
at 1.2 GHz for its first ~3.4 us of activity) while the first DMAs
stream. PSUM accumulation is fp32 throughout.
"""

import os
import sys

for _p in ("/opt/trn_rl_repo", "/root/.axon_site/_ro/trn_rl_repo"):
    if os.path.isdir(_p) and _p not in sys.path:
        sys.path.insert(0, _p)

import numpy as np
import ml_dtypes

BF16 = ml_dtypes.bfloat16

C = 512        # channels / model dim
T = 4096       # tokens (H*W)
S = 77         # context length
DCTX = 768     # context dim
HEADS = 8
DH = 64        # head dim
NT = 8         # token chunks
TC = T // NT   # 512 tokens per chunk
CT = C // 128  # 4 c-tiles
KT = DCTX // 128  # 6 context-dim tiles
NP = HEADS // 2   # 4 head pairs

_BUILT = None


def _build(dbg=False):
    import concourse.mybir as mybir
    import concourse.tile as tile
    from concourse import bacc
    from concourse.masks import make_identity

    f32 = mybir.dt.float32
    bf16 = mybir.dt.bfloat16
    AF = mybir.ActivationFunctionType

    nc = bacc.Bacc("TRN2", target_bir_lowering=False, debug=False, num_devices=8)

    # host-prearranged layouts: partition dim first, contiguous free dims
    X = nc.dram_tensor("x", [NT, 128, CT, TC], bf16, kind="ExternalInput")
    CTX = nc.dram_tensor("ctx", [S, DCTX], bf16, kind="ExternalInput")
    WQ = nc.dram_tensor("wq", [128, CT, C], bf16, kind="ExternalInput")
    WK = nc.dram_tensor("wk", [128, KT, C], bf16, kind="ExternalInput")
    WV = nc.dram_tensor("wv", [128, KT, C], bf16, kind="ExternalInput")
    WO = nc.dram_tensor("wo", [128, CT, C], bf16, kind="ExternalInput")
    BO = nc.dram_tensor("bo", [128, CT], f32, kind="ExternalInput")
    Y = nc.dram_tensor("y", [128, CT, NT, TC], bf16, kind="ExternalOutput")

    XR = X[:].rearrange("n p c t -> p n c t")

    with tile.TileContext(nc) as tc:
        with (
            tc.tile_pool(name="static", bufs=1) as st,
            tc.tile_pool(name="xin", bufs=3) as xp,
            tc.tile_pool(name="qt", bufs=2) as qp,
            tc.tile_pool(name="expsim", bufs=4) as ep,
            tc.tile_pool(name="outut", bufs=8) as op_,
            tc.tile_pool(name="bcast", bufs=3) as bp,
            tc.tile_pool(name="ysb", bufs=4) as yp,
            tc.tile_pool(name="ps_gemm", bufs=2, space="PSUM") as ps_g,
        ):
            # ---- DMA order = first-consumer order: wq + x chunk 0 feed the
            # hoisted chunk-0 Q projection (split in halves so the first
            # matmuls start after ~0.5 MB); ctx/wk/wv feed the K/V setup;
            # wo/bo are first needed one chunk later.
            wq = st.tile([128, CT, C], bf16, tag="wq")
            xs0 = xp.tile([128, CT, TC], bf16, tag="xs")
            for h2 in range(2):
                csl = slice(2 * h2, 2 * h2 + 2)
                nc.sync.dma_start(wq[:, csl, :], WQ[:][:, csl, :])
                nc.sync.dma_start(xs0[:, csl, :], XR[:, 0, csl, :])
            ctxs = st.tile([S, DCTX], bf16, tag="ctxs")
            nc.sync.dma_start(ctxs[:], CTX[:])
            wk = st.tile([128, KT, C], bf16, tag="wk")
            nc.sync.dma_start(wk[:], WK[:])
            wv = st.tile([128, KT, C], bf16, tag="wv")
            nc.sync.dma_start(wv[:], WV[:])
            wo = st.tile([128, CT, C], bf16, tag="wo")
            nc.sync.dma_start(wo[:], WO[:])
            bo = st.tile([128, CT], f32, tag="bo")
            nc.sync.dma_start(bo[:], BO[:])

            ident = st.tile([128, 128], bf16, tag="ident")
            make_identity(nc, ident[:])
            # all-ones selector stationaries (bf16): pair-broadcast rowsums.
            # ones_lo[j, c] = (c < 64), ones_hi[j, c] = (c >= 64)
            ones_lo = st.tile([S, 128], bf16, tag="ones_lo")
            nc.gpsimd.memset(ones_lo[:], 0.0)
            nc.gpsimd.memset(ones_lo[:, 0:DH], 1.0)
            ones_hi = st.tile([S, 128], bf16, tag="ones_hi")
            nc.gpsimd.memset(ones_hi[:], 0.0)
            nc.gpsimd.memset(ones_hi[:, DH:128], 1.0)

            # Q projection -> qT [128, 4, TC] (i on partitions); PSUM evac on
            # ACT (DVE carries recip + fused normalize in the main loop).
            def qproj(xs):
                qt = qp.tile([128, CT, TC], bf16, tag="qt")
                for it in range(CT):
                    pq = ps_g.tile([128, TC], f32, tag="pg")
                    for ct in range(CT):
                        nc.tensor.matmul(pq[:], wq[:, ct, it * 128:(it + 1) * 128],
                                         xs[:, ct, :],
                                         start=(ct == 0), stop=(ct == CT - 1))
                    nc.scalar.activation(qt[:, it, :], pq[:], AF.Copy)
                return qt

            # hoisted: chunk-0 Q projection runs while ctx/wk/wv still stream
            qt0 = qproj(xs0)

            # ---- setup: context transpose, K/V projections --------------------
            ctxT = st.tile([128, KT, S], bf16, tag="ctxT")
            ktp = st.tile([128, NP, S], bf16, tag="ktp")    # kT head-pairs
            # vpair[:, p, 0] = [v_2p | 0], vpair[:, p, 1] = [0 | v_2p+1]:
            # zero-padded M=128 stationaries so the AV pair accumulates into
            # one [128, TC] bank without col-tiling (quadrant 3 is invalid).
            vpair = st.tile([S, NP, 2, 128], bf16, tag="vpair")
            nc.gpsimd.memset(vpair[:], 0.0)
            with tc.tile_pool(name="ps_setup", bufs=1, space="PSUM") as ps_st:
                for ct in range(KT):
                    tp = ps_st.tile([128, S], bf16, tag=f"ctx_t{ct % 2}")
                    nc.tensor.transpose(tp[:], ctxs[:, ct * 128:(ct + 1) * 128], ident[0:S, 0:S])
                    nc.vector.tensor_copy(ctxT[:, ct, :], tp[:])
                kps = ps_st.tile([S, C], f32, tag="kproj")
                vps = ps_st.tile([S, C], f32, tag="vproj")
                for ct in range(KT):
                    nc.tensor.matmul(kps[:], ctxT[:, ct, :], wk[:, ct, :],
                                     start=(ct == 0), stop=(ct == KT - 1))
                for ct in range(KT):
                    nc.tensor.matmul(vps[:], ctxT[:, ct, :], wv[:, ct, :],
                                     start=(ct == 0), stop=(ct == KT - 1))
                ksb = st.tile([S, C], bf16, tag="ksb")
                nc.vector.tensor_copy(ksb[:], kps[:])
                for h in range(HEADS):
                    half = h % 2
                    nc.vector.tensor_copy(
                        vpair[:, h // 2, half, half * DH:half * DH + DH],
                        vps[:, h * DH:(h + 1) * DH])
                for h in range(HEADS):
                    tp = ps_st.tile([DH, S], bf16, tag=f"k_t{h % 2}")
                    nc.tensor.transpose(tp[:], ksb[:, h * DH:(h + 1) * DH], ident[0:S, 0:S])
                    base = (h % 2) * DH
                    nc.vector.tensor_copy(ktp[base:base + DH, h // 2, :], tp[:])

            with (
                tc.tile_pool(name="ps_sim", bufs=1, space="PSUM") as ps_sim,
                tc.tile_pool(name="ps_av", bufs=2, space="PSUM") as ps_av,
                tc.tile_pool(name="ps_rs", bufs=2, space="PSUM") as ps_rs,
            ):
                # ---- main loop over token chunks -----------------------------
                def oproj_group(t, ou, ct, tcs=slice(0, TC)):
                    n = tcs.stop - tcs.start
                    py = ps_g.tile([128, TC], f32, tag="pg")
                    for it in range(CT):
                        nc.tensor.matmul(py[:, 0:n], wo[:, it, ct * 128:(ct + 1) * 128],
                                         ou[it][:, tcs],
                                         start=(it == 0), stop=(it == CT - 1))
                    ys = yp.tile([128, TC], bf16, tag="ys")
                    if ct % 2 == 0:
                        nc.scalar.activation(ys[:, 0:n], py[:, 0:n], AF.Identity,
                                             bias=bo[:, ct:ct + 1])
                    else:
                        nc.vector.tensor_scalar_add(ys[:, 0:n], py[:, 0:n], bo[:, ct:ct + 1])
                    # Y writes ride the GpSimd queue: the Sync queue then only
                    # carries the x prefetches, which must never stall behind
                    # a ys-dependent write.
                    nc.gpsimd.dma_start(Y[:][:, ct, t, tcs], ys[:, 0:n])

                prev = None
                xs_next = xs0
                for t in range(NT):
                    xs = xs_next
                    if t + 1 < NT:
                        # prefetch next chunk's x one full chunk ahead
                        xs_next = xp.tile([128, CT, TC], bf16, tag="xs")
                        nc.sync.dma_start(xs_next[:], XR[:, t + 1, :, :])
                    qt = qt0 if t == 0 else qproj(xs)

                    # QK^T per pair into one 2-bank psum tile (row-tiled at
                    # bases 0/64: both matmuls run concurrently on the PE),
                    # one paired exp (scale 1/8 fused in ACT); pair-broadcast
                    # rowsums + AV pairs accumulate per pair; chunk t-1's O
                    # projection groups interleave as PE filler.
                    pairs = []
                    ogroups = list(range(CT)) if prev is not None else []

                    def emit_oproj_filler():
                        if ogroups:
                            oproj_group(t - 1, prev, ogroups.pop(0))

                    for p in range(NP):
                        psim = ps_sim.tile([128, 2, TC], f32, tag="psim")
                        for half in range(2):
                            base = half * DH
                            nc.tensor.matmul(psim[0:S, half, :], ktp[base:base + DH, p, :],
                                             qt[base:base + DH, p, :])
                        es = ep.tile([S, 2, TC], bf16, tag="exps")
                        nc.scalar.activation(es[:], psim[0:S, :, :], AF.Exp,
                                             scale=DH ** -0.5)
                        pav = ps_av.tile([128, TC], f32, tag="pav")
                        prs = ps_rs.tile([128, TC], f32, tag="prs")
                        pairs.append((pav, prs))
                        for half in range(2):
                            # pair-broadcast rowsum accumulate + AV pair
                            # (zero-padded M=128 stationaries accumulate into
                            # one psum bank)
                            nc.tensor.matmul(prs[:], ones_lo[:] if half == 0 else ones_hi[:],
                                             es[:, half, :], start=(half == 0),
                                             stop=(half == 1))
                            nc.tensor.matmul(pav[:], vpair[:, p, half, :],
                                             es[:, half, :], start=(half == 0),
                                             stop=(half == 1))
                        emit_oproj_filler()

                    # per-pair: bc_p = 1/psrs_p (PSUM -> SBUF, one DVE op),
                    # then fused evac/normalize: ou_p = pav_p * bc_p
                    ou = []
                    for p in range(NP):
                        pav, prs = pairs[p]
                        bcs = bp.tile([128, TC], f32, tag="bcs")
                        nc.vector.reciprocal_approx_fast(bcs[:], prs[:])
                        oup = op_.tile([128, TC], bf16, tag="ou")
                        nc.vector.tensor_tensor(
                            oup[:], pav[:], bcs[:],
                            mybir.AluOpType.mult)
                        ou.append(oup)

                    # leftover O-projection groups for chunk t-1
                    while ogroups:
                        emit_oproj_filler()

                    prev = ou

                # drain: O projection of the last chunk in half-width waves so
                # the bias/evac and Y writes overlap the remaining matmuls
                for ct in range(CT):
                    oproj_group(NT - 1, prev, ct, slice(0, TC // 2))
                for ct in range(CT):
                    oproj_group(NT - 1, prev, ct, slice(TC // 2, TC))

    nc.compile()
    return nc


def _get_nc():
    global _BUILT
    if _BUILT is None:
        _BUILT = _build()
    return _BUILT


def kernel(x, context, Wq, Wk, Wv, Wo, bo):
    from concourse.bass_utils import run_bass_kernel_spmd

    B = x.shape[0]
    assert B == 8 and x.shape == (8, C, 64, 64)
    nc = _get_nc()

    def wlayout(w, kt):  # [K, C] -> [128, kt, C] with row k = o*128 + p
        return np.ascontiguousarray(
            np.asarray(w, np.float32).astype(BF16).reshape(kt, 128, C).transpose(1, 0, 2))

    # x[b]: [C, T] -> [NT, 128, CT, TC] with c = ct*128 + p, t = n*TC + tc
    x8 = (np.asarray(x, np.float32).reshape(B, CT, 128, NT, TC)
          .transpose(0, 3, 2, 1, 4).astype(BF16))
    ctx8 = np.asarray(context, dtype=np.float32).astype(BF16)
    wq8 = wlayout(Wq, CT)
    wk8 = wlayout(Wk, KT)
    wv8 = wlayout(Wv, KT)
    wo8 = wlayout(Wo, CT)
    bo8 = np.ascontiguousarray(np.asarray(bo, np.float32).reshape(CT, 128).T)
    in_maps = [
        {
            "x": np.ascontiguousarray(x8[b]),
            "ctx": np.ascontiguousarray(ctx8[b]),
            "wq": wq8,
            "wk": wk8,
            "wv": wv8,
            "wo": wo8,
            "bo": bo8,
        }
        for b in range(8)
    ]
    res = run_bass_kernel_spmd(nc, in_maps, core_ids=list(range(8)))
    # y dram [128, CT, NT, TC] -> [C, T] with c = ct*128 + p
    out = []
    for r in res.results:
        yb = np.asarray(r["y"], np.float32)  # [128, CT, NT, TC]
        out.append(yb.transpose(1, 0, 2, 3).reshape(C, T).reshape(C, 64, 64))
    return np.stack(out)


# revision 17
# speedup vs baseline: 1.0536x; 1.0536x over previous
"""CrossAttention Trainium2 Bass kernel (v5).

Full inputs in, full output out. Data-parallel over batch: 8 batch elements
-> 8 NeuronCores; each core runs the whole cross-attention for one batch
element. Weights are replicated; no collectives.

Per-core computation (transposed domain end-to-end):
  x [512, 4096] (c-major)  -> qT = Wq.T @ x            [512(i), 4096(t)]
  ctx [77, 768]            -> k/v = ctxT.T @ Wk/Wv     [77(j), 512(i)]
  per head pair p: simT pair in one 2-bank PSUM tile   [77(j), 2, t]
      (the two QK matmuls are row-tiled at partition bases 0/64 and run
      concurrently on the PE); one paired ACT exp evacuates both halves.
  pair-broadcast rowsums: two accumulating matmuls with all-ones selector
      stationaries (ones_lo: cols 0-63, ones_hi: cols 64-127) produce
      psrs_p[c, t] = rowsum_{2p + c//64}[t] directly in broadcast layout.
  bc_p = 1/psrs_p  (DVE reciprocal_approx_fast, PSUM -> SBUF)
  AV pairs: pav_p [128, TC] = zero-padded pair bank (2 accumulating MMs)
  ou_p = pav_p * bc_p  (fused DVE tensor_tensor; PSUM evac + softmax
      normalization in one op; per-pair tiles so the O projection's first
      matmul only waits on pair 0)
  y = Wo.T @ ou + bo                                   [512(c), 4096(t)]

All matmul operands are bf16; inputs are cast AND pre-permuted to the
on-chip [partition, free] layouts host-side, so every DMA moves fully
contiguous >=4KB lines per partition (the v4 strided rearrange reads ran
at ~1/3 DMA rate and delayed the first matmul to 12 us). The output is
written bf16 in a chunk-major layout and unscrambled host-side. A short
burst of scratch matmuls pre-warms the PE's HAM clock gate (cold PE runs
at 1.2 GHz for its first ~3.4 us of activity) while the first DMAs
stream. PSUM accumulation is fp32 throughout.
"""

import os
import sys

for _p in ("/opt/trn_rl_repo", "/root/.axon_site/_ro/trn_rl_repo"):
    if os.path.isdir(_p) and _p not in sys.path:
        sys.path.insert(0, _p)

import numpy as np
import ml_dtypes

BF16 = ml_dtypes.bfloat16

C = 512        # channels / model dim
T = 4096       # tokens (H*W)
S = 77         # context length
DCTX = 768     # context dim
HEADS = 8
DH = 64        # head dim
NT = 8         # token chunks
TC = T // NT   # 512 tokens per chunk
CT = C // 128  # 4 c-tiles
KT = DCTX // 128  # 6 context-dim tiles
NP = HEADS // 2   # 4 head pairs

_BUILT = None


def _build(dbg=False):
    import concourse.mybir as mybir
    import concourse.tile as tile
    from concourse import bacc
    from concourse.masks import make_identity

    f32 = mybir.dt.float32
    bf16 = mybir.dt.bfloat16
    AF = mybir.ActivationFunctionType

    nc = bacc.Bacc("TRN2", target_bir_lowering=False, debug=False, num_devices=8)

    # host-prearranged layouts: partition dim first, contiguous free dims
    X = nc.dram_tensor("x", [NT, 128, CT, TC], bf16, kind="ExternalInput")
    CTX = nc.dram_tensor("ctx", [S, DCTX], bf16, kind="ExternalInput")
    WQ = nc.dram_tensor("wq", [128, CT, C], bf16, kind="ExternalInput")
    WK = nc.dram_tensor("wk", [128, KT, C], bf16, kind="ExternalInput")
    WV = nc.dram_tensor("wv", [128, KT, C], bf16, kind="ExternalInput")
    WO = nc.dram_tensor("wo", [128, CT, C], bf16, kind="ExternalInput")
    BO = nc.dram_tensor("bo", [128, CT], f32, kind="ExternalInput")
    Y = nc.dram_tensor("y", [128, CT, NT, TC], bf16, kind="ExternalOutput")

    XR = X[:].rearrange("n p c t -> p n c t")

    with tile.TileContext(nc) as tc:
        with (
            tc.tile_pool(name="static", bufs=1) as st,
            tc.tile_pool(name="xin", bufs=3) as xp,
            tc.tile_pool(name="qt", bufs=2) as qp,
            tc.tile_pool(name="expsim", bufs=4) as ep,
            tc.tile_pool(name="outut", bufs=8) as op_,
            tc.tile_pool(name="bcast", bufs=3) as bp,
            tc.tile_pool(name="ysb", bufs=4) as yp,
            tc.tile_pool(name="ps_gemm", bufs=2, space="PSUM") as ps_g,
        ):
            # ---- DMA order = first-consumer order: wq + x chunk 0 feed the
            # hoisted chunk-0 Q projection (split in halves so the first
            # matmuls start after ~0.5 MB); ctx/wk/wv feed the K/V setup;
            # wo/bo are first needed one chunk later.
            wq = st.tile([128, CT, C], bf16, tag="wq")
            xs0 = xp.tile([128, CT, TC], bf16, tag="xs")
            for h2 in range(2):
                csl = slice(2 * h2, 2 * h2 + 2)
                nc.sync.dma_start(wq[:, csl, :], WQ[:][:, csl, :])
                nc.sync.dma_start(xs0[:, csl, :], XR[:, 0, csl, :])
            ctxs = st.tile([S, DCTX], bf16, tag="ctxs")
            nc.sync.dma_start(ctxs[:], CTX[:])
            wk = st.tile([128, KT, C], bf16, tag="wk")
            nc.sync.dma_start(wk[:], WK[:])
            wv = st.tile([128, KT, C], bf16, tag="wv")
            nc.sync.dma_start(wv[:], WV[:])
            wo = st.tile([128, CT, C], bf16, tag="wo")
            nc.sync.dma_start(wo[:], WO[:])
            bo = st.tile([128, CT], f32, tag="bo")
            nc.sync.dma_start(bo[:], BO[:])

            ident = st.tile([128, 128], bf16, tag="ident")
            make_identity(nc, ident[:])
            # all-ones selector stationaries (bf16): pair-broadcast rowsums.
            # ones_lo[j, c] = (c < 64), ones_hi[j, c] = (c >= 64)
            ones_lo = st.tile([S, 128], bf16, tag="ones_lo")
            nc.gpsimd.memset(ones_lo[:], 0.0)
            nc.gpsimd.memset(ones_lo[:, 0:DH], 1.0)
            ones_hi = st.tile([S, 128], bf16, tag="ones_hi")
            nc.gpsimd.memset(ones_hi[:], 0.0)
            nc.gpsimd.memset(ones_hi[:, DH:128], 1.0)

            # ---- PE pre-warm: ~10 scratch matmuls (~4.3 us at the cold 1.2
            # GHz clock) while the first DMAs stream (input DMA takes ~12 us
            # end-to-end, so this is free). HAM un-throttles to 2.4 GHz after
            # ~3.4 us of sustained PE activity, so the real matmuls start
            # warm. No data dependencies: operates on a memset tile.
            scr = st.tile([128, TC], bf16, tag="scr")
            nc.vector.memset(scr[:], 0.0)
            with tc.tile_pool(name="ps_warm", bufs=1, space="PSUM") as ps_w:
                pw = ps_w.tile([128, TC], f32, tag="pw")
                for _ in range(10):
                    nc.tensor.matmul(pw[:], scr[:, 0:128], scr[:])

            # Q projection -> qT [128, 4, TC] (i on partitions); PSUM evac on
            # ACT (DVE carries recip + fused normalize in the main loop).
            def qproj(xs):
                qt = qp.tile([128, CT, TC], bf16, tag="qt")
                for it in range(CT):
                    pq = ps_g.tile([128, TC], f32, tag="pg")
                    for ct in range(CT):
                        nc.tensor.matmul(pq[:], wq[:, ct, it * 128:(it + 1) * 128],
                                         xs[:, ct, :],
                                         start=(ct == 0), stop=(ct == CT - 1))
                    nc.scalar.activation(qt[:, it, :], pq[:], AF.Copy)
                return qt

            # hoisted: chunk-0 Q projection runs while ctx/wk/wv still stream
            qt0 = qproj(xs0)

            # ---- setup: context transpose, K/V projections --------------------
            ctxT = st.tile([128, KT, S], bf16, tag="ctxT")
            ktp = st.tile([128, NP, S], bf16, tag="ktp")    # kT head-pairs
            # vpair[:, p, 0] = [v_2p | 0], vpair[:, p, 1] = [0 | v_2p+1]:
            # zero-padded M=128 stationaries so the AV pair accumulates into
            # one [128, TC] bank without col-tiling (quadrant 3 is invalid).
            vpair = st.tile([S, NP, 2, 128], bf16, tag="vpair")
            nc.gpsimd.memset(vpair[:], 0.0)
            with tc.tile_pool(name="ps_setup", bufs=1, space="PSUM") as ps_st:
                for ct in range(KT):
                    tp = ps_st.tile([128, S], bf16, tag=f"ctx_t{ct % 2}")
                    nc.tensor.transpose(tp[:], ctxs[:, ct * 128:(ct + 1) * 128], ident[0:S, 0:S])
                    nc.vector.tensor_copy(ctxT[:, ct, :], tp[:])
                kps = ps_st.tile([S, C], f32, tag="kproj")
                vps = ps_st.tile([S, C], f32, tag="vproj")
                for ct in range(KT):
                    nc.tensor.matmul(kps[:], ctxT[:, ct, :], wk[:, ct, :],
                                     start=(ct == 0), stop=(ct == KT - 1))
                for ct in range(KT):
                    nc.tensor.matmul(vps[:], ctxT[:, ct, :], wv[:, ct, :],
                                     start=(ct == 0), stop=(ct == KT - 1))
                ksb = st.tile([S, C], bf16, tag="ksb")
                nc.vector.tensor_copy(ksb[:], kps[:])
                for h in range(HEADS):
                    half = h % 2
                    nc.vector.tensor_copy(
                        vpair[:, h // 2, half, half * DH:half * DH + DH],
                        vps[:, h * DH:(h + 1) * DH])
                for h in range(HEADS):
                    tp = ps_st.tile([DH, S], bf16, tag=f"k_t{h % 2}")
                    nc.tensor.transpose(tp[:], ksb[:, h * DH:(h + 1) * DH], ident[0:S, 0:S])
                    base = (h % 2) * DH
                    nc.vector.tensor_copy(ktp[base:base + DH, h // 2, :], tp[:])

            with (
                tc.tile_pool(name="ps_sim", bufs=1, space="PSUM") as ps_sim,
                tc.tile_pool(name="ps_av", bufs=2, space="PSUM") as ps_av,
                tc.tile_pool(name="ps_rs", bufs=2, space="PSUM") as ps_rs,
            ):
                # ---- main loop over token chunks -----------------------------
                def oproj_group(t, ou, ct, tcs=slice(0, TC)):
                    n = tcs.stop - tcs.start
                    py = ps_g.tile([128, TC], f32, tag="pg")
                    for it in range(CT):
                        nc.tensor.matmul(py[:, 0:n], wo[:, it, ct * 128:(ct + 1) * 128],
                                         ou[it][:, tcs],
                                         start=(it == 0), stop=(it == CT - 1))
                    ys = yp.tile([128, TC], bf16, tag="ys")
                    if ct % 2 == 0:
                        nc.scalar.activation(ys[:, 0:n], py[:, 0:n], AF.Identity,
                                             bias=bo[:, ct:ct + 1])
                    else:
                        nc.vector.tensor_scalar_add(ys[:, 0:n], py[:, 0:n], bo[:, ct:ct + 1])
                    # Y writes ride the GpSimd queue: the Sync queue then only
                    # carries the x prefetches, which must never stall behind
                    # a ys-dependent write.
                    nc.gpsimd.dma_start(Y[:][:, ct, t, tcs], ys[:, 0:n])

                prev = None
                xs_next = xs0
                for t in range(NT):
                    xs = xs_next
                    if t + 1 < NT:
                        # prefetch next chunk's x one full chunk ahead
                        xs_next = xp.tile([128, CT, TC], bf16, tag="xs")
                        nc.sync.dma_start(xs_next[:], XR[:, t + 1, :, :])
                    qt = qt0 if t == 0 else qproj(xs)

                    # QK^T per pair into one 2-bank psum tile (row-tiled at
                    # bases 0/64: both matmuls run concurrently on the PE),
                    # one paired exp (scale 1/8 fused in ACT); pair-broadcast
                    # rowsums + AV pairs accumulate per pair; chunk t-1's O
                    # projection groups interleave as PE filler.
                    pairs = []
                    ogroups = list(range(CT)) if prev is not None else []

                    def emit_oproj_filler():
                        if ogroups:
                            oproj_group(t - 1, prev, ogroups.pop(0))

                    for p in range(NP):
                        psim = ps_sim.tile([128, 2, TC], f32, tag="psim")
                        for half in range(2):
                            base = half * DH
                            nc.tensor.matmul(psim[0:S, half, :], ktp[base:base + DH, p, :],
                                             qt[base:base + DH, p, :])
                        es = ep.tile([S, 2, TC], bf16, tag="exps")
                        nc.scalar.activation(es[:], psim[0:S, :, :], AF.Exp,
                                             scale=DH ** -0.5)
                        pav = ps_av.tile([128, TC], f32, tag="pav")
                        prs = ps_rs.tile([128, TC], f32, tag="prs")
                        pairs.append((pav, prs))
                        for half in range(2):
                            # pair-broadcast rowsum accumulate + AV pair
                            # (zero-padded M=128 stationaries accumulate into
                            # one psum bank)
                            nc.tensor.matmul(prs[:], ones_lo[:] if half == 0 else ones_hi[:],
                                             es[:, half, :], start=(half == 0),
                                             stop=(half == 1))
                            nc.tensor.matmul(pav[:], vpair[:, p, half, :],
                                             es[:, half, :], start=(half == 0),
                                             stop=(half == 1))
                        emit_oproj_filler()

                    # per-pair: bc_p = 1/psrs_p (PSUM -> SBUF, one DVE op),
                    # then fused evac/normalize: ou_p = pav_p * bc_p
                    ou = []
                    for p in range(NP):
                        pav, prs = pairs[p]
                        bcs = bp.tile([128, TC], f32, tag="bcs")
                        nc.vector.reciprocal_approx_fast(bcs[:], prs[:])
                        oup = op_.tile([128, TC], bf16, tag="ou")
                        nc.vector.tensor_tensor(
                            oup[:], pav[:], bcs[:],
                            mybir.AluOpType.mult)
                        ou.append(oup)

                    # leftover O-projection groups for chunk t-1
                    while ogroups:
                        emit_oproj_filler()

                    prev = ou

                # drain: O projection of the last chunk
                for ct in range(CT):
                    oproj_group(NT - 1, prev, ct)

    nc.compile()
    return nc


def _get_nc():
    global _BUILT
    if _BUILT is None:
        _BUILT = _build()
    return _BUILT


def kernel(x, context, Wq, Wk, Wv, Wo, bo):
    from concourse.bass_utils import run_bass_kernel_spmd

    B = x.shape[0]
    assert B == 8 and x.shape == (8, C, 64, 64)
    nc = _get_nc()

    def wlayout(w, kt):  # [K, C] -> [128, kt, C] with row k = o*128 + p
        return np.ascontiguousarray(
            np.asarray(w, np.float32).astype(BF16).reshape(kt, 128, C).transpose(1, 0, 2))

    # x[b]: [C, T] -> [NT, 128, CT, TC] with c = ct*128 + p, t = n*TC + tc
    x8 = (np.asarray(x, np.float32).reshape(B, CT, 128, NT, TC)
          .transpose(0, 3, 2, 1, 4).astype(BF16))
    ctx8 = np.asarray(context, dtype=np.float32).astype(BF16)
    wq8 = wlayout(Wq, CT)
    wk8 = wlayout(Wk, KT)
    wv8 = wlayout(Wv, KT)
    wo8 = wlayout(Wo, CT)
    bo8 = np.ascontiguousarray(np.asarray(bo, np.float32).reshape(CT, 128).T)
    in_maps = [
        {
            "x": np.ascontiguousarray(x8[b]),
            "ctx": np.ascontiguousarray(ctx8[b]),
            "wq": wq8,
            "wk": wk8,
            "wv": wv8,
            "wo": wo8,
            "bo": bo8,
        }
        for b in range(8)
    ]
    res = run_bass_kernel_spmd(nc, in_maps, core_ids=list(range(8)))
    # y dram [128, CT, NT, TC] -> [C, T] with c = ct*128 + p
    out = []
    for r in res.results:
        yb = np.asarray(r["y"], np.float32)  # [128, CT, NT, TC]
        out.append(yb.transpose(1, 0, 2, 3).reshape(C, T).reshape(C, 64, 64))
    return np.stack(out)
